# revision 3
# baseline (speedup 1.0000x reference)
"""Trainium2 Bass kernel for ConvSelfAttention (B=4, C=128, W=H=64).

Reference computation (per batch b, with N = W*H = 4096):
    q = wq @ x + bq ; k = wk @ x + bk ; v = wv @ x + bv        # [C, N]
    S[n, m] = (q[:, n] . k[:, m]) / sqrt(C)
    A = softmax(S, axis=m)                                     # [N, N]
    out[c, n] = sum_m v[c, m] A[n, m]
    y = wo @ out + bo
    result = gamma * y + x

Sharding: 8 cores = 4 batches x 2 halves of the attention-row dim n.
Each core holds full x[b] (for k, v) and computes out[:, n_slice].

v2 architecture (vs the fp16 v1 baseline, 101us):
  - Scores: fp8e4m3 DoubleRow matmuls in a [64, 2, .] channel-split
    layout (contraction 2x64) -- ~2x the fp16 column rate.  The scale
    A8 = 8/ln2 (and 1/sqrt(C)) is folded into wq host-side, so PSUM
    scores arrive pre-scaled for the fp8 exp bit trick.
  - exp (the engine wall): split between Scalar (native Exp, fp8 out,
    scale=1/A8, bias=-ln SC) and Vector (Schraudolph bit trick:
    int8(round(max(S' + B8, 0))) reinterpreted as fp8e4m3 ~ exp(S)/SC).
    Weights are normalized by their own approximate denominator, so
    common-mode error cancels; the residual-dominated output tolerates
    the rest (tolerance 2e-2, attention is ~12% of output norm).
  - P@V and the tp>=11 denominator part: fp8 DoubleRow (contraction
    2x128/pass).  tp<=10 denominator: fp16 accumulators on the
    otherwise-idle Pool engine, folded into the same PSUM accumulation
    group at the end via fp16 ones-matmuls.
  - PSUM (8 banks): scores 2x[128,2,512] (4) + P@V out [128,1024] (2) +
    one 2-slot ring shared by projections -> denominators -> out-proj,
    time-sliced by the schedule.

Host-side folding: 1/sqrt(C) and A8 into wq, bq; bk dropped (cancels in
softmax); bv folded into bo via wo (softmax rows sum to 1); gamma into
wo and bo_eff.
"""

import math
import os
import sys

import numpy as np

if "/opt/trn_rl_repo" not in sys.path:
    sys.path.insert(0, "/opt/trn_rl_repo")

B, C, W, H = 4, 128, 64, 64
N = W * H            # 4096
HALF = N // 2        # 2048 n-columns per core
CHUNK = 512
MT = N // 128        # 32 m-tiles
TP = MT // 2         # 16 m-tile pairs

LN2 = math.log(2.0)
A8 = 8.0 / LN2       # folded into wq host-side; scores arrive as A8*S
SC = 32.0            # weights are exp(S)/SC (fp8e4m3 range guard)
B8 = 8.0 * 7.0 - A8 * math.log(SC) - 0.22   # Schraudolph bias (round conv)

# DVE-exp superjob assignment per block (g = 2*tp + h in 0..31).
DVE_G = (
    frozenset(range(0, 32, 2)),                    # block 0: 16 (DVE also copies)
    frozenset(range(1, 27, 2)),                    # block 1: 13
)
POOL_TP_MAX = 10     # tp 0..10 denominator on Pool, tp 11..15 on PE (DR)

_BUILT = {}


def _build():
    if "nc" in _BUILT:
        return _BUILT["nc"]

    import concourse.bass as bass
    from concourse import bacc, mybir
    from concourse.tile import TileContext

    f32 = mybir.dt.float32
    f16 = mybir.dt.float16
    f8 = mybir.dt.float8e4
    i8 = mybir.dt.int8
    DR = mybir.MatmulPerfMode.DoubleRow
    ADD = mybir.AluOpType.add
    MAX = mybir.AluOpType.max

    nc = bacc.Bacc("TRN2", target_bir_lowering=False)

    x_d = nc.dram_tensor("x", [C, N], f16, kind="ExternalInput")
    wp_d = nc.dram_tensor("wpack", [C, 4 * C], f16, kind="ExternalInput")
    bp_d = nc.dram_tensor("bpack", [C, 2], f32, kind="ExternalInput")
    y_d = nc.dram_tensor("y", [C, HALF], f32, kind="ExternalOutput")

    with TileContext(nc) as tc:
        with (
            tc.tile_pool(name="consts", bufs=1) as consts,
            tc.tile_pool(name="bigs", bufs=1) as bigs,
            tc.tile_pool(name="exps", bufs=6) as exps,
            tc.tile_pool(name="accs", bufs=2) as accs,
            tc.tile_pool(name="smalls", bufs=2) as smalls,
            tc.tile_pool(name="outs", bufs=3) as outs,
            tc.tile_pool(name="xs", bufs=4) as xs,
            tc.tile_pool(name="ps_s", bufs=2, space="PSUM") as pp_s,
            tc.tile_pool(name="ps_o", bufs=1, space="PSUM") as pp_o,
            tc.tile_pool(name="ps_x", bufs=2, space="PSUM") as pp_x,
        ):
            # ---- constants & inputs -------------------------------------
            wp_sb = consts.tile([C, 4 * C], f16, tag="wp")
            bp_sb = consts.tile([C, 2], f32, tag="bp")
            ones8 = consts.tile([128, 2, C], f8, tag="on8")
            ones16 = consts.tile([128, C], f16, tag="on16")
            bias_e = consts.tile([128, 1], f32, tag="be")
            dummy = consts.tile([C, 1], f16, tag="dmy")

            wq_sb = wp_sb[:, 0:C]          # s*A8 folded
            wk_sb = wp_sb[:, C:2 * C]
            wv_sb = wp_sb[:, 2 * C:3 * C]
            wo_sb = wp_sb[:, 3 * C:4 * C]  # gamma folded
            bo_sb = bp_sb[:, 1:2]

            # Warm the ACT exp table while DMAs are in flight.
            nc.vector.memset(bias_e, -math.log(SC))
            nc.vector.memset(dummy, 0.0)
            nc.scalar.activation(
                dummy, dummy, mybir.ActivationFunctionType.Exp,
                bias=bias_e[:, 0:1],
            )
            nc.vector.memset(ones8, 1.0)
            nc.vector.memset(ones16, 1.0)

            k8 = bigs.tile([64, 2, N], f8, tag="k8")
            q8 = bigs.tile([64, 2, HALF], f8, tag="q8")
            vT8 = bigs.tile([128, TP, 2, C], f8, tag="vT8")
            outN = bigs.tile([C, HALF], f16, tag="outN")

            # x pieces: 4 x 1024 cols
            nc.sync.dma_start(out=wp_sb, in_=wp_d[:, :])
            nc.gpsimd.dma_start(out=bp_sb, in_=bp_d[:, :])
            piece = []
            for pi in range(4):
                xp = xs.tile([C, 1024], f16, tag="xp", name=f"xp{pi}")
                eng = nc.scalar if pi < 2 else nc.sync
                eng.dma_start(out=xp, in_=x_d[:, bass.ds(1024 * pi, 1024)])
                piece.append(xp)

            def xh(cn, w):
                pi, off = divmod(cn, 1024)
                assert off + w <= 1024
                return piece[pi][:, bass.ds(off, w)]

            # ---- production units (512 m-cols each) ---------------------
            # Channel-split [64, 2, .] layouts: two 64-wide projection
            # matmuls per chunk (j halves both land on partitions 0..63).
            def kunit(c):
                for j in range(2):
                    ps = pp_x.tile([64, CHUNK], f32, tag="x",
                                   name=f"psk{c}_{j}")
                    nc.tensor.matmul(
                        ps, wp_sb[:, bass.ds(C + 64 * j, 64)],
                        xh(c * CHUNK, CHUNK), start=True, stop=True,
                    )
                    nc.scalar.activation(
                        k8[:, j, bass.ds(c * CHUNK, CHUNK)], ps,
                        mybir.ActivationFunctionType.Copy,
                    )

            def qunit(c):
                for j in range(2):
                    ps = pp_x.tile([64, CHUNK], f32, tag="x",
                                   name=f"psq{c}_{j}")
                    nc.tensor.matmul(
                        ps, wp_sb[:, bass.ds(64 * j, 64)],
                        xh(c * CHUNK, CHUNK), start=True, stop=True,
                    )
                    nc.vector.tensor_scalar_add(
                        q8[:, j, bass.ds(c * CHUNK, CHUNK)], ps,
                        bp_sb[bass.ds(64 * j, 64), 0:1],
                    )

            def vunit(g):
                ps = pp_x.tile([128, CHUNK], f32, tag="x", name=f"psv{g}")
                for tt in range(4):
                    nc.tensor.matmul(
                        ps[:, bass.ts(tt, 128)],
                        xh(g * CHUNK + tt * 128, 128),
                        wv_sb, start=True, stop=True,
                    )
                nc.vector.tensor_copy(
                    vT8[:, bass.ds(2 * g, 2), :, :],
                    ps.rearrange("p (t c) -> p t c", c=C),
                )

            # prefetch the units the first superjobs need
            kunit(0)
            qunit(0)
            vunit(0)
            bundles = [lambda: qunit(1)]
            for c in range(1, 8):
                bundles.append(lambda c=c: kunit(c))
                bundles.append(lambda g=c: vunit(g))
            bundles.append(lambda: qunit(2))
            bundles.append(lambda: qunit(3))

            # ---- attention main loop ------------------------------------
            jobs = [(ci, tp, h) for ci in range(2) for tp in range(TP)
                    for h in range(2)]

            def emit_scores(ci, tp, h):
                ps = pp_s.tile([128, 2, CHUNK], f32, tag="s",
                               name=f"s{ci}_{tp}_{h}")
                n0 = ci * 1024 + h * CHUNK
                for j in range(2):
                    nc.tensor.matmul(
                        ps[:, j, :],
                        k8[:, :, bass.ds((2 * tp + j) * 128, 128)],
                        q8[:, :, bass.ds(n0, CHUNK)],
                        start=True, stop=True, perf_mode=DR,
                    )
                return ps

            pending = {jobs[0]: emit_scores(*jobs[0])}

            psum_o = None
            acc16 = None
            dens = {}
            for i, (ci, tp, h) in enumerate(jobs):
                g = i % 32
                if g == 0:
                    psum_o = pp_o.tile([128, 1024], f32, tag="o",
                                       name=f"o{ci}")
                    acc16 = accs.tile([128, 2, 1024], f16, tag="a",
                                      name=f"a{ci}")

                # production bundles first: their drains queue ahead of
                # this superjob's exp on the same engines.
                if bundles and i >= 1:
                    bundles.pop(0)()

                ps_s = pending.pop((ci, tp, h))
                e_t = exps.tile([128, 2, CHUNK], i8, tag="e",
                                name=f"e{ci}_{tp}_{h}")
                ef = e_t.bitcast(f8)
                if g in DVE_G[ci]:
                    nc.vector.tensor_scalar(
                        e_t, ps_s, B8, 0.0, op0=ADD, op1=MAX,
                    )
                else:
                    nc.scalar.activation(
                        ef, ps_s, mybir.ActivationFunctionType.Exp,
                        bias=bias_e[:, 0:1], scale=1.0 / A8,
                    )
                if i + 1 < len(jobs):
                    nj = jobs[i + 1]
                    pending[nj] = emit_scores(*nj)

                # P@V (DoubleRow, contraction 2x128 per pass)
                nc.tensor.matmul(
                    psum_o[:, bass.ds(h * CHUNK, CHUNK)],
                    vT8[:, tp, :, :], ef,
                    start=(tp == 0), stop=(tp == TP - 1), perf_mode=DR,
                )

                # denominator
                a_sl = acc16[:, :, bass.ds(h * CHUNK, CHUNK)]
                if tp == 0:
                    nc.gpsimd.tensor_copy(a_sl, ef)
                elif tp <= POOL_TP_MAX:
                    nc.gpsimd.tensor_tensor(a_sl, a_sl, ef, op=ADD)
                else:
                    if tp == POOL_TP_MAX + 1:
                        dens[h] = pp_x.tile([128, CHUNK], f32, tag="x",
                                            name=f"den{ci}_{h}")
                    nc.tensor.matmul(
                        dens[h], ones8, ef,
                        start=(tp == POOL_TP_MAX + 1), stop=False,
                        perf_mode=DR,
                    )
                    if tp == TP - 1:
                        for j in range(2):
                            nc.tensor.matmul(
                                dens[h], ones16,
                                acc16[:, j, bass.ds(h * CHUNK, CHUNK)],
                                start=False, stop=(j == 1),
                            )

                # ---- block epilogue ----
                if g == 31:
                    for hh in range(2):
                        cn = ci * 1024 + hh * CHUNK
                        rb = smalls.tile([128, CHUNK], f32, tag="rb",
                                         name=f"rb{ci}_{hh}")
                        nc.vector.reciprocal_approx_fast(rb, dens[hh])
                        nc.vector.tensor_mul(
                            outN[:, bass.ds(cn, CHUNK)],
                            psum_o[:, bass.ds(hh * CHUNK, CHUNK)], rb,
                        )
                        ps_y = pp_x.tile([128, CHUNK], f32, tag="x",
                                         name=f"psy{ci}_{hh}")
                        nc.tensor.matmul(
                            ps_y, wo_sb, outN[:, bass.ds(cn, CHUNK)],
                            start=True, stop=True,
                        )
                        t2 = outs.tile([128, CHUNK], f32, tag="t2",
                                       name=f"t2{ci}_{hh}")
                        nc.vector.scalar_tensor_tensor(
                            t2, ps_y, bo_sb, xh(cn, CHUNK),
                            op0=ADD, op1=ADD,
                        )
                        nc.sync.dma_start(
                            out=y_d[:, bass.ds(cn, CHUNK)], in_=t2
                        )

    nc.compile()
    _BUILT["nc"] = nc
    return nc


def _make_in_maps(inputs):
    x = np.asarray(inputs["x"], np.float32)
    wq = np.asarray(inputs["wq"], np.float32)
    bq = np.asarray(inputs["bq"], np.float32)
    wk = np.asarray(inputs["wk"], np.float32)
    wv = np.asarray(inputs["wv"], np.float32)
    bv = np.asarray(inputs["bv"], np.float32)
    wo = np.asarray(inputs["wo"], np.float32)
    bo = np.asarray(inputs["bo"], np.float32)
    gamma = float(np.asarray(inputs["gamma"], np.float32)[0])

    s = (1.0 / math.sqrt(C)) * A8
    wpack = np.ascontiguousarray(np.hstack([
        (wq * s).T, wk.T, wv.T, (wo * gamma).T,
    ]).astype(np.float16))
    bpack = np.ascontiguousarray(np.stack([
        bq * s, gamma * (wo @ bv + bo),
    ], axis=1).astype(np.float32))

    xf = x.reshape(B, C, N).astype(np.float16)
    in_maps = []
    for core in range(8):
        b, half = core // 2, core % 2
        own = xf[b][:, half * HALF:(half + 1) * HALF]
        oth = xf[b][:, (1 - half) * HALF:(2 - half) * HALF]
        in_maps.append({
            "x": np.ascontiguousarray(np.hstack([own, oth])),
            "wpack": wpack,
            "bpack": bpack,
        })
    return in_maps


def _gather(results):
    out = np.empty((B, C, N), np.float32)
    for core in range(8):
        b, half = core // 2, core % 2
        out[b][:, half * HALF:(half + 1) * HALF] = results[core]["y"]
    return out.reshape(B, C, W, H)


def run(inputs, trace=False):
    """Run on the 8 NeuronCores; returns (output, exec_time_ns_or_None)."""
    from concourse.bass_utils import run_bass_kernel_spmd

    nc = _build()
    in_maps = _make_in_maps(inputs)
    res = run_bass_kernel_spmd(nc, in_maps, core_ids=list(range(8)), trace=trace)
    return _gather(res.results), res.exec_time_ns


def kernel(**inputs):
    out, _ = run(inputs)
    return out
